# revision 23
# baseline (speedup 1.0000x reference)
"""GPT-2-ish forward (B=4, T=1024, D=768, H=12, L=2, V=50257) on 8 trn2 cores.

Sharding: core pair (2b, 2b+1) handles batch b. Within the pair the trunk is
sequence-split: core 2b+h owns tokens [512h, 512h+512). Per layer each core
computes Q/K/V for its own tokens, the K/V halves are exchanged with an
in-pair AllGather, and attention/proj/MLP run on own tokens only. The causal
structure is uniform across cores (same program); per-core masks (input data)
zero the score blocks a core's half doesn't need. After the final LN the x
halves are all-gathered and each core runs lm_head over all 1024 tokens for
its 25600-column vocab half.

On-device layout: activations [features, tokens]. Attention scores use
zero-padded per-head K stationaries (full 128-partition contraction so FWL
stays on); att @ V is computed with V as the stationary operand so the output
lands directly in [feature, token] layout, with a ones-column in V providing
the softmax denominator. LayerNorm stats (ones-vector matmuls) are
interleaved into the producer loops (proj/fc2) to keep the PE stream
continuous; normalization uses two rank-1 broadcast matmuls and per-feature
g/b on the scalar engine. Weights are host-pre-blocked so each SBUF weight
tile is one contiguous DMA. All matmuls bf16 with fp32 PSUM; residual fp32;
logits evicted fp16 and upcast on host.
"""

import numpy as np
import ml_dtypes
from contextlib import ExitStack

import concourse.bass as bass
from concourse import bacc
import concourse.mybir as mybir
import concourse.tile as tile
from concourse.bass_utils import run_bass_kernel_spmd

BF16 = mybir.dt.bfloat16
F32 = mybir.dt.float32
F16 = mybir.dt.float16
AF = mybir.ActivationFunctionType
ALU = mybir.AluOpType

V = 50257
VPAD = 51200          # 2 * 25600
VSH = VPAD // 2       # per-core vocab shard
D = 768
H = 12
HD = 64
L = 2
T = 1024
B = 4
TH = 512              # tokens per core (half sequence)
EPS = 1e-5
NKT = D // 128        # 6 feature tiles over D
NTT = T // 128        # 8 token tiles full seq
NTH = TH // 128       # 4 token tiles own half
NVC = VSH // 512      # 50 lm vocab chunks per core
RG = [[0, 1], [2, 3], [4, 5], [6, 7]]

TRACE = False
LAST_RESULT = None

_G = {}


class LnState:
    """LayerNorm stats accumulated incrementally as producer loops write x."""

    def __init__(self, tc, nc, tag, pool, small, g_d, b_d, xbfp):
        self.tag = tag
        self.st = pool.tile([33, TH], F32, tag="st", name=f"st_{tag}")
        self.xbf = [xbfp.tile([128, TH], BF16, tag=f"xbf{i}", name=f"xbf{i}_{tag}")
                    for i in range(NKT)]
        self.next_kt = 0
        # prefetch g/b at state creation (well before the finish needs them)
        self.gb_sb = small.tile([128, 2 * NKT], F32, tag="gb", name=f"gb_{tag}")
        nc.sync.dma_start(self.gb_sb[:, 0:NKT],
                          g_d.rearrange("(t p) -> p t", p=128))
        nc.sync.dma_start(self.gb_sb[:, NKT:2 * NKT],
                          b_d.rearrange("(t p) -> p t", p=128))

    def stat_kt(self, nc, scratch, x_tile):
        kt = self.next_kt
        self.next_kt += 1
        xbf = self.xbf[kt]
        sq = scratch.tile([128, TH], BF16, tag="ln_sq", name="ln_sq")
        nc.scalar.copy(xbf, x_tile)
        nc.vector.tensor_mul(sq, xbf, xbf)
        nc.tensor.matmul(self.st[0:1, :], _G["ones_bf"], xbf,
                         start=(kt == 0), stop=(kt == NKT - 1))
        nc.tensor.matmul(self.st[32:33, :], _G["ones_bf"], sq,
                         start=(kt == 0), stop=(kt == NKT - 1))


def ln_finish(tc, nc, st, xt, out_tiles, small, scratch, ab_pool):
    """Consume accumulated stats, write bf16 out_tiles = LN(x)*g+b."""
    assert st.next_kt == NKT
    gb_sb = st.gb_sb
    mean = small.tile([1, TH], F32, tag="mean", name="mean", bufs=1)
    var = small.tile([1, TH], F32, tag="var", name="var", bufs=1)
    rstd = small.tile([1, TH], F32, tag="rstd", name="rstd", bufs=1)
    mr_bf = small.tile([1, TH], BF16, tag="mr_bf", name="mr_bf", bufs=1)
    rstd_bf = small.tile([1, TH], BF16, tag="rstd_bf", name="rstd_bf", bufs=1)
    nc.vector.tensor_scalar_mul(mean, st.st[0:1, :], 1.0 / D)
    nc.vector.tensor_mul(var, mean, mean)
    nc.vector.scalar_tensor_tensor(var, st.st[32:33, :], 1.0 / D, var,
                                   op0=ALU.mult, op1=ALU.subtract)
    nc.scalar.activation(var, var, AF.Sqrt, bias=_G["eps_sb"])
    nc.vector.reciprocal_approx_fast(rstd, var)
    nc.vector.tensor_copy(rstd_bf, rstd)
    nc.vector.tensor_mul(mr_bf, mean, rstd)
    # R = bcast(rstd) [128,TH]; M = bcast(mean*rstd) [128,TH]
    R = ab_pool.tile([128, TH], F32, tag="R", name="R")
    M = ab_pool.tile([128, TH], F32, tag="M", name="M")
    nc.tensor.matmul(R, _G["ones_col128"], rstd_bf, start=True, stop=True)
    nc.tensor.matmul(M, _G["ones_col128"], mr_bf, start=True, stop=True)
    R_sb = small.tile([128, TH], BF16, tag="R_sb", name="R_sb", bufs=1)
    M_sb = small.tile([128, TH], BF16, tag="M_sb", name="M_sb", bufs=1)
    nc.scalar.copy(R_sb, R)
    nc.scalar.copy(M_sb, M)
    for kt in range(NKT):
        tmp = scratch.tile([128, TH], BF16, tag="lntmp", name="lntmp")
        nc.vector.tensor_mul(tmp, st.xbf[kt], R_sb)
        nc.vector.tensor_sub(tmp, tmp, M_sb)
        # out = (x*rstd - mean*rstd) * g + b   (per-partition g/b)
        nc.scalar.activation(out_tiles[kt], tmp, AF.Identity,
                             bias=gb_sb[:, NKT + kt:NKT + kt + 1],
                             scale=gb_sb[:, kt:kt + 1])


def build_bass():
    nc = bacc.Bacc(None, target_bir_lowering=False, num_devices=8)
    # ---- DRAM I/O (per-core shard views; weights host-pre-blocked) ----
    xT_d = nc.dram_tensor("xT", [D, TH], F32, kind="ExternalInput")
    qkw_d = nc.dram_tensor("qkw", [L, 12, 128, NKT, 128], BF16,
                           kind="ExternalInput")
    vw_d = nc.dram_tensor("vw", [L, D, D], BF16, kind="ExternalInput")
    pw_d = nc.dram_tensor("pw", [L, D, D], BF16, kind="ExternalInput")
    fcw_d = nc.dram_tensor("fcw", [L, 24, 128, NKT, 128], BF16,
                           kind="ExternalInput")
    fc2w_d = nc.dram_tensor("fc2w", [L, 6, 128, 24, 128], BF16,
                            kind="ExternalInput")
    qkb_d = nc.dram_tensor("qkb", [L, 2 * D], F32, kind="ExternalInput")
    vb_d = nc.dram_tensor("vb", [L, D], BF16, kind="ExternalInput")
    pb_d = nc.dram_tensor("pb", [L, D], F32, kind="ExternalInput")
    fcb_d = nc.dram_tensor("fcb", [L, 4 * D], F32, kind="ExternalInput")
    fc2b_d = nc.dram_tensor("fc2b", [L, D], F32, kind="ExternalInput")
    ln_d = nc.dram_tensor("lnp", [L, 4, D], F32, kind="ExternalInput")
    lnf_d = nc.dram_tensor("lnf", [2, D], F32, kind="ExternalInput")
    mask_d = nc.dram_tensor("mask", [NTT, 128, TH], BF16, kind="ExternalInput")
    lmw_d = nc.dram_tensor("lmw", [NVC, 128, NKT, 512], BF16,
                           kind="ExternalInput")
    out_d = nc.dram_tensor("out", [T, VSH], F16, kind="ExternalOutput")

    with tile.TileContext(nc) as tc, ExitStack() as octx:
        singles = octx.enter_context(tc.tile_pool(name="singles", bufs=1))
        resid = octx.enter_context(tc.tile_pool(name="resid", bufs=1))
        dram = octx.enter_context(tc.tile_pool(name="dram", bufs=1, space="DRAM"))
        lnsp = octx.enter_context(tc.tile_pool(name="lnsp", bufs=2, space="PSUM"))
        scratch = octx.enter_context(tc.tile_pool(name="scratch", bufs=2))
        xbfp = octx.enter_context(tc.tile_pool(name="xbfp", bufs=1))
        small = octx.enter_context(tc.tile_pool(name="small", bufs=2))

        # constants
        ones_bf = singles.tile([128, 1], BF16)
        nc.vector.memset(ones_bf, 1.0)
        ones_col128 = singles.tile([1, 128], BF16)   # rank-1 bcast stationary
        nc.vector.memset(ones_col128, 1.0)
        ones_row = singles.tile([1, 512], BF16)
        nc.vector.memset(ones_row, 1.0)
        eps_sb = singles.tile([1, 1], F32)
        nc.vector.memset(eps_sb, EPS)
        _G["ones_bf"] = ones_bf
        _G["ones_col128"] = ones_col128
        _G["eps_sb"] = eps_sb

        # residual stream (own half), fp32, resident
        xt = [resid.tile([128, TH], F32, tag=f"xt{i}", name=f"xt{i}")
              for i in range(NKT)]
        for kt in range(NKT):
            nc.sync.dma_start(xt[kt], xT_d[kt * 128:(kt + 1) * 128, :])

        # padded K stationaries: kp0[pr] rows 0:64 = head 2pr K, rows 64: zero
        kp0 = [resid.tile([128, T], BF16, tag=f"kp0_{i}", name=f"kp0_{i}")
               for i in range(6)]
        kp1 = [resid.tile([128, T], BF16, tag=f"kp1_{i}", name=f"kp1_{i}")
               for i in range(6)]
        for pr in range(6):
            nc.gpsimd.memset(kp0[pr][64:128, :], 0.0)
            nc.gpsimd.memset(kp1[pr][0:64, :], 0.0)
        # V natural [tokens, 12 heads, 64+1] with ones column (full seq)
        v_aug = [resid.tile([128, H, HD + 1], BF16, tag=f"vaug{i}", name=f"vaug{i}")
                 for i in range(NTT)]
        for tt in range(NTT):
            nc.gpsimd.memset(v_aug[tt][:, :, HD:HD + 1], 1.0)

        mask_sb = singles.tile([128, NTT, TH], BF16)

        # LN1 of layer 0: stats directly after the x DMAs
        ln_next = LnState(tc, nc, "l0a", lnsp, small, ln_d[0][0], ln_d[0][1], xbfp)
        for kt in range(NKT):
            ln_next.stat_kt(nc, scratch, xt[kt])

        for l in range(L):
            with ExitStack() as lctx:
                lnpool = lctx.enter_context(tc.tile_pool(name=f"ln{l}", bufs=1))
                wpool = lctx.enter_context(tc.tile_pool(name=f"w{l}", bufs=3))
                biasp = lctx.enter_context(tc.tile_pool(name=f"bias{l}", bufs=1))

                qkb_sb = biasp.tile([128, 12], F32)
                nc.sync.dma_start(qkb_sb, qkb_d[l].rearrange("(t p) -> p t", p=128))
                vbbf_sb = biasp.tile([1, D], BF16)
                nc.sync.dma_start(vbbf_sb, vb_d[l].rearrange("(o d) -> o d", o=1))
                pb_sb = biasp.tile([128, 6], F32)
                nc.sync.dma_start(pb_sb, pb_d[l].rearrange("(t p) -> p t", p=128))
                fcb_sb = biasp.tile([128, 24], F32)
                nc.sync.dma_start(fcb_sb, fcb_d[l].rearrange("(t p) -> p t", p=128))
                fc2b_sb = biasp.tile([128, 6], F32)
                nc.sync.dma_start(fc2b_sb, fc2b_d[l].rearrange("(t p) -> p t", p=128))

                # ---------- LN1 finish ----------
                h_bf = [lnpool.tile([128, TH], BF16, tag=f"hbf{i}", name=f"hbf{i}")
                        for i in range(NKT)]
                with tc.tile_pool(name=f"ab{l}a", bufs=1, space="PSUM") as ab_ps:
                    ln_finish(tc, nc, ln_next, xt, h_bf, small, scratch, ab_ps)

                # ---------- K own-half -> staging -> AllGather ----------
                k_own = lnpool.tile([128, 6, TH], BF16, tag="k_own", name="k_own")
                kin_b = dram.tile([128, 6, TH], BF16, tag=f"kin{l}", name=f"kin{l}")
                kout_b = dram.tile([2, 128, 6, TH], BF16, tag=f"kout{l}",
                                   name=f"kout{l}")
                with tc.tile_pool(name=f"qkps{l}", bufs=3, space="PSUM") as qkps:
                    for pr in range(6):
                        f = 6 + pr
                        wt = wpool.tile([128, NKT, 128], BF16, tag="qkw_t",
                                        name="qkw_t")
                        nc.sync.dma_start(wt, qkw_d[l, f])
                        ps = qkps.tile([128, TH], F32, tag="qkps", name="qkps")
                        for kt in range(NKT):
                            nc.tensor.matmul(ps, wt[:, kt, :], h_bf[kt],
                                             start=(kt == 0), stop=(kt == NKT - 1))
                        nc.scalar.activation(k_own[:, pr, :], ps, AF.Identity,
                                             bias=qkb_sb[:, f:f + 1])
                    nc.sync.dma_start(kin_b, k_own)
                    nc.gpsimd.collective_compute(
                        "AllGather", ALU.bypass, replica_groups=RG,
                        ins=[kin_b[:]], outs=[kout_b[:]])

                    # ---------- V own-half -> staging -> AllGather ----------
                    v_own = lnpool.tile([128, NTH, D], BF16, tag="v_own",
                                        name="v_own")
                    vin_b = dram.tile([128, NTH, D], BF16, tag=f"vin{l}",
                                      name=f"vin{l}")
                    vout_b = dram.tile([2, 128, NTH, D], BF16, tag=f"vout{l}",
                                       name=f"vout{l}")
                    vw_sb = [wpool.tile([128, D], BF16, tag=f"vw{i}",
                                        name=f"vw{i}", bufs=1)
                             for i in range(NKT)]
                    for kt in range(NKT):
                        nc.sync.dma_start(vw_sb[kt],
                                          vw_d[l][kt * 128:(kt + 1) * 128, :])
                    for tt in range(NTH):
                        for vc in range(2):
                            vs = slice(vc * 384, (vc + 1) * 384)
                            ps = qkps.tile([128, 384], F32, tag="vps", name="vps")
                            for kt in range(NKT):
                                nc.tensor.matmul(
                                    ps, h_bf[kt][:, tt * 128:(tt + 1) * 128],
                                    vw_sb[kt][:, vs],
                                    start=(kt == 0), stop=False)
                            nc.tensor.matmul(ps, ones_row[:, 0:128],
                                             vbbf_sb[:, vs],
                                             start=False, stop=True)
                            nc.vector.tensor_copy(v_own[:, tt, vs], ps)
                    nc.sync.dma_start(vin_b, v_own)
                    nc.gpsimd.collective_compute(
                        "AllGather", ALU.bypass, replica_groups=RG,
                        ins=[vin_b[:]], outs=[vout_b[:]])

                    # ---------- Q own-half ----------
                    q_sb = [lnpool.tile([128, TH], BF16, tag=f"q{i}", name=f"q{i}")
                            for i in range(6)]
                    for pr in range(6):
                        wt = wpool.tile([128, NKT, 128], BF16, tag="qkw_t",
                                        name="qkw_t")
                        nc.sync.dma_start(wt, qkw_d[l, pr])
                        ps = qkps.tile([128, TH], F32, tag="qkps", name="qkps")
                        for kt in range(NKT):
                            nc.tensor.matmul(ps, wt[:, kt, :], h_bf[kt],
                                             start=(kt == 0), stop=(kt == NKT - 1))
                        nc.scalar.activation(q_sb[pr], ps, AF.Identity,
                                             bias=qkb_sb[:, pr:pr + 1])

                if l == 0:
                    # masks are first needed by the scores below; deferring the
                    # DMA keeps startup queues free for x/weights
                    nc.sync.dma_start(mask_sb, mask_d.rearrange("j p q -> p j q"))

                # K readback into padded stationaries (both halves, uniform)
                for pr in range(6):
                    for rk in range(2):
                        cs = slice(rk * TH, (rk + 1) * TH)
                        nc.sync.dma_start(kp0[pr][0:64, cs],
                                          kout_b[rk, 0:64, pr, :])
                        nc.sync.dma_start(kp1[pr][64:128, cs],
                                          kout_b[rk, 64:128, pr, :])
                # V readback (both halves)
                for tt in range(NTT):
                    nc.sync.dma_start(
                        v_aug[tt][:, :, 0:HD],
                        vout_b[tt // NTH, :, tt % NTH, :]
                        .rearrange("p (h d) -> p h d", d=HD))

                # ---------- attention per head-pair ----------
                attoT = [lnpool.tile([128, TH], BF16, tag=f"attoT{i}",
                                     name=f"attoT{i}")
                         for i in range(NKT)]
                with tc.tile_pool(name=f"sps{l}", bufs=3, space="PSUM") as sps, \
                     tc.tile_pool(name=f"ops{l}", bufs=2, space="PSUM") as ops, \
                     tc.tile_pool(name=f"bps{l}", bufs=1, space="PSUM") as bps, \
                     tc.tile_pool(name=f"attp{l}", bufs=2) as attp:
                    for pr in range(6):
                        attT = [[attp.tile([128, TH], BF16, tag=f"attT{hh}_{kt}",
                                           name=f"attT{hh}_{kt}")
                                 for kt in range(NTT)] for hh in range(2)]
                        for kt in range(NTT):
                            ks = slice(kt * 128, (kt + 1) * 128)
                            for hh, kp in ((0, kp0), (1, kp1)):
                                ps = sps.tile([128, TH], F32, tag="sps",
                                              name="sps")
                                nc.tensor.matmul(ps, kp[pr][:, ks], q_sb[pr],
                                                 start=True, stop=True)
                                dst = attT[hh][kt]
                                nc.scalar.activation(dst, ps, AF.Exp,
                                                     scale=0.125)
                                nc.vector.tensor_mul(dst, dst,
                                                     mask_sb[:, kt, :])
                        for hh in range(2):
                            h = 2 * pr + hh
                            po = ops.tile([128, TH], F32, tag="ops", name="ops")
                            for kt in range(NTT):
                                nc.tensor.matmul(
                                    po[0:HD + 1, :],
                                    v_aug[kt][:, h, :], attT[hh][kt],
                                    start=(kt == 0), stop=(kt == NTT - 1))
                            den_sb = scratch.tile([1, TH], F32, tag="den_sb",
                                                  name="den_sb")
                            r_sb = scratch.tile([1, TH], F32, tag="r_sb",
                                                name="r_sb")
                            rb = scratch.tile([1, TH], BF16, tag="rb", name="rb")
                            nc.scalar.copy(den_sb, po[HD:HD + 1, :])
                            nc.vector.reciprocal_approx_fast(r_sb, den_sb)
                            nc.vector.tensor_copy(rb, r_sb)
                            bc = bps.tile([64, TH], F32, tag="bc", name="bc")
                            nc.tensor.matmul(bc, ones_col128[:, 0:64], rb,
                                             start=True, stop=True)
                            bc_sb = scratch.tile([64, TH], BF16, tag="bc_sb",
                                                 name="bc_sb")
                            nc.scalar.copy(bc_sb, bc)
                            nc.vector.tensor_mul(
                                attoT[pr][hh * 64:(hh + 1) * 64, :],
                                po[0:HD, :], bc_sb)

                # ---------- proj + residual (LN2 stats interleaved) ----------
                pw_sb = [wpool.tile([128, D], BF16, tag=f"pw{i}", name=f"pw{i}",
                                    bufs=1)
                         for i in range(NKT)]
                for kt in range(NKT):
                    nc.sync.dma_start(pw_sb[kt], pw_d[l][kt * 128:(kt + 1) * 128, :])
                ln2 = LnState(tc, nc, f"l{l}b", lnsp, small,
                              ln_d[l][2], ln_d[l][3], xbfp)
                with tc.tile_pool(name=f"pps{l}", bufs=4, space="PSUM") as pps:
                    for ot in range(NKT):
                        ps = pps.tile([128, TH], F32, tag="pps", name="pps")
                        for kt in range(NKT):
                            nc.tensor.matmul(
                                ps, pw_sb[kt][:, ot * 128:(ot + 1) * 128],
                                attoT[kt],
                                start=(kt == 0), stop=(kt == NKT - 1))
                        nc.vector.scalar_tensor_tensor(
                            xt[ot], ps, pb_sb[:, ot:ot + 1],
                            xt[ot], op0=ALU.add, op1=ALU.add)
                        if ot >= 1:
                            ln2.stat_kt(nc, scratch, xt[ot - 1])
                    ln2.stat_kt(nc, scratch, xt[NKT - 1])

                # ---------- LN2 finish + MLP (next-LN stats interleaved) ----
                h2in = [lnpool.tile([128, TH], BF16, tag=f"h2bf{i}",
                                    name=f"h2bf{i}")
                        for i in range(NKT)]
                with tc.tile_pool(name=f"ab{l}b", bufs=1, space="PSUM") as ab_ps:
                    ln_finish(tc, nc, ln2, xt, h2in, small, scratch, ab_ps)

                if l + 1 < L:
                    ln_next = LnState(tc, nc, f"l{l + 1}a", lnsp, small,
                                      ln_d[l + 1][0], ln_d[l + 1][1], xbfp)
                else:
                    ln_next = LnState(tc, nc, "lf", lnsp, small,
                                      lnf_d[0], lnf_d[1], xbfp)
                with tc.tile_pool(name=f"mlpps{l}", bufs=3, space="PSUM") as mlpps, \
                     tc.tile_pool(name=f"h2p{l}", bufs=1) as h2p:
                    h2c = [h2p.tile([128, TH], BF16, tag=f"h2c{f}", name=f"h2c{f}")
                           for f in range(24)]
                    for f in range(24):
                        wt = wpool.tile([128, NKT, 128], BF16, tag="fcw_t",
                                        name="fcw_t")
                        nc.sync.dma_start(wt, fcw_d[l, f])
                        ps = mlpps.tile([128, TH], F32, tag="fcps", name="fcps")
                        for kt in range(NKT):
                            nc.tensor.matmul(ps, wt[:, kt, :], h2in[kt],
                                             start=(kt == 0), stop=(kt == NKT - 1))
                        nc.scalar.activation(h2c[f], ps, AF.Gelu_apprx_tanh,
                                             bias=fcb_sb[:, f:f + 1])
                    for ot in range(NKT):
                        wt = wpool.tile([128, 24, 128], BF16, tag="fc2w_t",
                                        name="fc2w_t", bufs=2)
                        nc.sync.dma_start(wt, fc2w_d[l, ot])
                        ps = mlpps.tile([128, TH], F32, tag="fc2ps", name="fc2ps")
                        for kt in range(24):
                            nc.tensor.matmul(ps, wt[:, kt, :], h2c[kt],
                                             start=(kt == 0), stop=(kt == 23))
                        nc.vector.scalar_tensor_tensor(
                            xt[ot], ps, fc2b_sb[:, ot:ot + 1],
                            xt[ot], op0=ALU.add, op1=ALU.add)
                        if ot >= 1:
                            ln_next.stat_kt(nc, scratch, xt[ot - 1])
                    ln_next.stat_kt(nc, scratch, xt[NKT - 1])

        # ---------- final LN + AllGather + lm_head ----------
        with ExitStack() as fctx:
            lnpool = fctx.enter_context(tc.tile_pool(name="lnfp", bufs=1))
            xf_own = [lnpool.tile([128, TH], BF16, tag=f"xfo{i}", name=f"xfo{i}")
                      for i in range(NKT)]
            lmwp = fctx.enter_context(tc.tile_pool(name="lmw", bufs=3))
            wt_pre = {}
            with tc.tile_pool(name="abf", bufs=1, space="PSUM") as ab_ps:
                ln_finish(tc, nc, ln_next, xt, xf_own, small, scratch, ab_ps)
            xin_b = dram.tile([128, NKT, TH], BF16, tag="xin", name="xin")
            xout_b = dram.tile([2, 128, NKT, TH], BF16, tag="xout", name="xout")
            for kt in range(NKT):
                nc.sync.dma_start(xin_b[:, kt, :], xf_own[kt])
            nc.gpsimd.collective_compute(
                "AllGather", ALU.bypass, replica_groups=RG,
                ins=[xin_b[:]], outs=[xout_b[:]])
            xf_bf = [lnpool.tile([128, T], BF16, tag=f"xf{i}", name=f"xf{i}")
                     for i in range(NKT)]
            for kt in range(NKT):
                for rk in range(2):
                    nc.sync.dma_start(xf_bf[kt][:, rk * TH:(rk + 1) * TH],
                                      xout_b[rk, :, kt, :])

            with tc.tile_pool(name="lmps", bufs=4, space="PSUM") as lmps, \
                 tc.tile_pool(name="lmev", bufs=4) as lmev:
                for vc in range(NVC):
                    if vc in wt_pre:
                        wt = wt_pre.pop(vc)
                    else:
                        wt = lmwp.tile([128, NKT, 512], BF16, tag="lmw_t",
                                       name="lmw_t")
                        nc.sync.dma_start(wt, lmw_d[vc])
                    for tt in range(NTT):
                        ps = lmps.tile([128, 512], F32, tag="lmps", name="lmps")
                        for kt in range(NKT):
                            nc.tensor.matmul(
                                ps, xf_bf[kt][:, tt * 128:(tt + 1) * 128],
                                wt[:, kt, :],
                                start=(kt == 0), stop=(kt == NKT - 1))
                        ev = lmev.tile([128, 512], F16, tag="lmev", name="lmev")
                        if tt % 2 == 0:
                            nc.scalar.copy(ev, ps)
                        else:
                            nc.vector.tensor_copy(ev, ps)
                        nc.sync.dma_start(
                            out_d[tt * 128:(tt + 1) * 128,
                                  vc * 512:(vc + 1) * 512], ev)
    nc.finalize()
    return nc


_NC_CACHE = None


def _get_nc():
    global _NC_CACHE
    if _NC_CACHE is None:
        _NC_CACHE = build_bass()
    return _NC_CACHE


def _block_w(w, no, nt):
    """[IN=nt*128, OUT=no*128] -> [no, 128, nt, 128] blocked for contiguous DMA:
    out[o, p, t, c] = w[t*128+p, o*128+c]."""
    IN, OUT = w.shape
    return np.ascontiguousarray(
        w.reshape(nt, 128, no, 128).transpose(2, 1, 0, 3))


def make_in_maps(idx, layer_num, wte, wpe, ln1_g, ln1_b, attn_w, attn_b, proj_w,
                 proj_b, ln2_g, ln2_b, fc_w, fc_b, fc2_w, fc2_b, lnf_g, lnf_b, lm_w):
    bf = ml_dtypes.bfloat16
    idx = np.asarray(idx)
    f32 = np.float32
    wte = np.asarray(wte, f32)
    wpe = np.asarray(wpe, f32)
    x0 = wte[idx] + wpe[:T]                      # [B,T,D] fp32 host embedding

    attn_w = np.asarray(attn_w, f32)
    qkw = np.stack([_block_w(attn_w[l, :, :2 * D], 12, NKT) for l in range(L)]
                   ).astype(bf)
    vw = np.ascontiguousarray(attn_w[:, :, 2 * D:]).astype(bf)
    pw = np.asarray(proj_w, f32).astype(bf)
    fcw = np.stack([_block_w(np.asarray(fc_w, f32)[l], 24, NKT)
                    for l in range(L)]).astype(bf)
    fc2w = np.stack([_block_w(np.asarray(fc2_w, f32)[l], 6, 24)
                     for l in range(L)]).astype(bf)
    qkb = np.ascontiguousarray(np.asarray(attn_b, f32)[:, :2 * D])
    vb = np.ascontiguousarray(np.asarray(attn_b, f32)[:, 2 * D:]).astype(bf)
    lnp = np.stack([np.asarray(ln1_g, f32), np.asarray(ln1_b, f32),
                    np.asarray(ln2_g, f32), np.asarray(ln2_b, f32)], axis=1)
    lnf = np.stack([np.asarray(lnf_g, f32), np.asarray(lnf_b, f32)], axis=0)

    lmw_pad = np.zeros((D, VPAD), f32)
    lmw_pad[:, :V] = np.asarray(lm_w, f32)

    # causal mask blocks per half h: mask[j][kk, qq] = (128*j + kk) <= (512*h + qq)
    jj = np.arange(NTT)[:, None, None] * 128 + np.arange(128)[None, :, None]
    qq = np.arange(TH)[None, None, :]
    masks = [(jj <= h * TH + qq).astype(bf) for h in range(2)]

    x0T = [np.ascontiguousarray(x0[b].T) for b in range(B)]
    # lm_w halves blocked: [NVC, 128, NKT, 512]; lmw[vc, p, t, c] =
    # w[t*128+p, vc*512+c]
    lmw_halves = []
    for h in range(2):
        wh = lmw_pad[:, h * VSH:(h + 1) * VSH]
        lmw_halves.append(np.ascontiguousarray(
            wh.reshape(NKT, 128, NVC, 512).transpose(2, 1, 0, 3)).astype(bf))

    in_maps = []
    for core in range(8):
        b = core // 2
        h = core % 2
        in_maps.append(dict(
            xT=np.ascontiguousarray(x0T[b][:, h * TH:(h + 1) * TH]),
            qkw=qkw, vw=vw, pw=pw, fcw=fcw, fc2w=fc2w,
            qkb=qkb, vb=vb, pb=np.asarray(proj_b, f32),
            fcb=np.asarray(fc_b, f32), fc2b=np.asarray(fc2_b, f32),
            lnp=lnp, lnf=lnf, mask=masks[h],
            lmw=lmw_halves[h],
        ))
    return in_maps


def kernel(**inputs):
    global LAST_RESULT
    in_maps = make_in_maps(**inputs)
    nc = _get_nc()
    res = run_bass_kernel_spmd(nc, in_maps, core_ids=list(range(8)), trace=TRACE)
    LAST_RESULT = res

    logits = np.empty((B, T, V), np.float32)
    for b in range(B):
        lo = res.results[2 * b]["out"].astype(np.float32)
        hi = res.results[2 * b + 1]["out"].astype(np.float32)
        logits[b, :, :VSH] = lo
        logits[b, :, VSH:] = hi[:, :V - VSH]
    return logits


# revision 27
# speedup vs baseline: 1.0144x; 1.0144x over previous
"""GPT-2-ish forward (B=4, T=1024, D=768, H=12, L=2, V=50257) on 8 trn2 cores.

Sharding: core pair (2b, 2b+1) handles batch b. Within the pair the trunk is
sequence-split: core 2b+h owns tokens [512h, 512h+512). Per layer each core
computes Q/K/V for its own tokens, the K/V halves are exchanged with an
in-pair AllGather, and attention/proj/MLP run on own tokens only. The causal
structure is uniform across cores (same program); per-core masks (input data)
zero the score blocks a core's half doesn't need. After the final LN the x
halves are all-gathered and each core runs lm_head over all 1024 tokens for
its 25600-column vocab half.

On-device layout: activations [features, tokens]. Attention scores use
zero-padded per-head K stationaries (full 128-partition contraction so FWL
stays on); att @ V is computed with V as the stationary operand so the output
lands directly in [feature, token] layout, with a ones-column in V providing
the softmax denominator. LayerNorm stats (ones-vector matmuls) are
interleaved into the producer loops (proj/fc2) to keep the PE stream
continuous; normalization uses two rank-1 broadcast matmuls and per-feature
g/b on the scalar engine. Weights are host-pre-blocked so each SBUF weight
tile is one contiguous DMA. All matmuls bf16 with fp32 PSUM; residual fp32;
logits evicted fp16 and upcast on host.
"""

import numpy as np
import ml_dtypes
from contextlib import ExitStack

import concourse.bass as bass
from concourse import bacc
import concourse.mybir as mybir
import concourse.tile as tile
from concourse.bass_utils import run_bass_kernel_spmd

BF16 = mybir.dt.bfloat16
F32 = mybir.dt.float32
F16 = mybir.dt.float16
AF = mybir.ActivationFunctionType
ALU = mybir.AluOpType

V = 50257
VPAD = 51200          # 2 * 25600
VSH = VPAD // 2       # per-core vocab shard
D = 768
H = 12
HD = 64
L = 2
T = 1024
B = 4
TH = 512              # tokens per core (half sequence)
EPS = 1e-5
NKT = D // 128        # 6 feature tiles over D
NTT = T // 128        # 8 token tiles full seq
NTH = TH // 128       # 4 token tiles own half
NVC = VSH // 512      # 50 lm vocab chunks per core
RG = [[0, 1], [2, 3], [4, 5], [6, 7]]

TRACE = False
LAST_RESULT = None

_G = {}


class LnState:
    """LayerNorm stats accumulated incrementally as producer loops write x."""

    def __init__(self, tc, nc, tag, pool, small, g_d, b_d, xbfp):
        self.tag = tag
        self.st = pool.tile([33, TH], F32, tag="st", name=f"st_{tag}")
        self.xbf = [xbfp.tile([128, TH], BF16, tag=f"xbf{i}", name=f"xbf{i}_{tag}")
                    for i in range(NKT)]
        self.next_kt = 0
        # prefetch g/b at state creation (well before the finish needs them)
        self.gb_sb = small.tile([128, 2 * NKT], F32, tag="gb", name=f"gb_{tag}")
        nc.sync.dma_start(self.gb_sb[:, 0:NKT],
                          g_d.rearrange("(t p) -> p t", p=128))
        nc.sync.dma_start(self.gb_sb[:, NKT:2 * NKT],
                          b_d.rearrange("(t p) -> p t", p=128))

    def stat_kt(self, nc, scratch, x_tile):
        kt = self.next_kt
        self.next_kt += 1
        xbf = self.xbf[kt]
        sq = scratch.tile([128, TH], BF16, tag="ln_sq", name="ln_sq")
        nc.scalar.copy(xbf, x_tile)
        nc.vector.tensor_mul(sq, xbf, xbf)
        nc.tensor.matmul(self.st[0:1, :], _G["ones_bf"], xbf,
                         start=(kt == 0), stop=(kt == NKT - 1))
        nc.tensor.matmul(self.st[32:33, :], _G["ones_bf"], sq,
                         start=(kt == 0), stop=(kt == NKT - 1))


def ln_finish(tc, nc, st, xt, out_tiles, small, scratch, ab_pool):
    """Consume accumulated stats, write bf16 out_tiles = LN(x)*g+b."""
    assert st.next_kt == NKT
    gb_sb = st.gb_sb
    mean = small.tile([1, TH], F32, tag="mean", name="mean", bufs=1)
    var = small.tile([1, TH], F32, tag="var", name="var", bufs=1)
    rstd = small.tile([1, TH], F32, tag="rstd", name="rstd", bufs=1)
    mr_bf = small.tile([1, TH], BF16, tag="mr_bf", name="mr_bf", bufs=1)
    rstd_bf = small.tile([1, TH], BF16, tag="rstd_bf", name="rstd_bf", bufs=1)
    nc.vector.tensor_scalar_mul(mean, st.st[0:1, :], 1.0 / D)
    nc.vector.tensor_mul(var, mean, mean)
    nc.vector.scalar_tensor_tensor(var, st.st[32:33, :], 1.0 / D, var,
                                   op0=ALU.mult, op1=ALU.subtract)
    nc.scalar.activation(var, var, AF.Sqrt, bias=_G["eps_sb"])
    nc.vector.reciprocal_approx_fast(rstd, var)
    nc.vector.tensor_copy(rstd_bf, rstd)
    nc.vector.tensor_mul(mr_bf, mean, rstd)
    # R = bcast(rstd) [128,TH]; M = bcast(mean*rstd) [128,TH]
    R = ab_pool.tile([128, TH], F32, tag="R", name="R")
    M = ab_pool.tile([128, TH], F32, tag="M", name="M")
    nc.tensor.matmul(R, _G["ones_col128"], rstd_bf, start=True, stop=True)
    nc.tensor.matmul(M, _G["ones_col128"], mr_bf, start=True, stop=True)
    R_sb = small.tile([128, TH], BF16, tag="R_sb", name="R_sb", bufs=1)
    M_sb = small.tile([128, TH], BF16, tag="M_sb", name="M_sb", bufs=1)
    nc.scalar.copy(R_sb, R)
    nc.scalar.copy(M_sb, M)
    for kt in range(NKT):
        tmp = scratch.tile([128, TH], BF16, tag="lntmp", name="lntmp")
        nc.vector.tensor_mul(tmp, st.xbf[kt], R_sb)
        nc.vector.tensor_sub(tmp, tmp, M_sb)
        # out = (x*rstd - mean*rstd) * g + b   (per-partition g/b)
        nc.scalar.activation(out_tiles[kt], tmp, AF.Identity,
                             bias=gb_sb[:, NKT + kt:NKT + kt + 1],
                             scale=gb_sb[:, kt:kt + 1])


def build_bass():
    nc = bacc.Bacc(None, target_bir_lowering=False, num_devices=8)
    # ---- DRAM I/O (per-core shard views; weights host-pre-blocked) ----
    xT_d = nc.dram_tensor("xT", [D, TH], F32, kind="ExternalInput")
    qkw_d = nc.dram_tensor("qkw", [L, 12, 128, NKT, 128], BF16,
                           kind="ExternalInput")
    vw_d = nc.dram_tensor("vw", [L, D, D], BF16, kind="ExternalInput")
    pw_d = nc.dram_tensor("pw", [L, D, D], BF16, kind="ExternalInput")
    fcw_d = nc.dram_tensor("fcw", [L, 24, 128, NKT, 128], BF16,
                           kind="ExternalInput")
    fc2w_d = nc.dram_tensor("fc2w", [L, 6, 128, 24, 128], BF16,
                            kind="ExternalInput")
    qkb_d = nc.dram_tensor("qkb", [L, 2 * D], F32, kind="ExternalInput")
    vb_d = nc.dram_tensor("vb", [L, D], BF16, kind="ExternalInput")
    pb_d = nc.dram_tensor("pb", [L, D], F32, kind="ExternalInput")
    fcb_d = nc.dram_tensor("fcb", [L, 4 * D], F32, kind="ExternalInput")
    fc2b_d = nc.dram_tensor("fc2b", [L, D], F32, kind="ExternalInput")
    ln_d = nc.dram_tensor("lnp", [L, 4, D], F32, kind="ExternalInput")
    lnf_d = nc.dram_tensor("lnf", [2, D], F32, kind="ExternalInput")
    mask_d = nc.dram_tensor("mask", [NTT, 128, TH], BF16, kind="ExternalInput")
    lmw_d = nc.dram_tensor("lmw", [NVC, 128, NKT, 512], BF16,
                           kind="ExternalInput")
    out_d = nc.dram_tensor("out", [T, VSH], F16, kind="ExternalOutput")

    with tile.TileContext(nc) as tc, ExitStack() as octx:
        singles = octx.enter_context(tc.tile_pool(name="singles", bufs=1))
        resid = octx.enter_context(tc.tile_pool(name="resid", bufs=1))
        dram = octx.enter_context(tc.tile_pool(name="dram", bufs=1, space="DRAM"))
        lnsp = octx.enter_context(tc.tile_pool(name="lnsp", bufs=2, space="PSUM"))
        scratch = octx.enter_context(tc.tile_pool(name="scratch", bufs=2))
        xbfp = octx.enter_context(tc.tile_pool(name="xbfp", bufs=1))
        small = octx.enter_context(tc.tile_pool(name="small", bufs=2))

        # constants
        ones_bf = singles.tile([128, 1], BF16)
        nc.vector.memset(ones_bf, 1.0)
        ones_col128 = singles.tile([1, 128], BF16)   # rank-1 bcast stationary
        nc.vector.memset(ones_col128, 1.0)
        ones_row = singles.tile([1, 512], BF16)
        nc.vector.memset(ones_row, 1.0)
        eps_sb = singles.tile([1, 1], F32)
        nc.vector.memset(eps_sb, EPS)
        _G["ones_bf"] = ones_bf
        _G["ones_col128"] = ones_col128
        _G["eps_sb"] = eps_sb

        # residual stream (own half), fp32, resident
        xt = [resid.tile([128, TH], F32, tag=f"xt{i}", name=f"xt{i}")
              for i in range(NKT)]
        for kt in range(NKT):
            nc.sync.dma_start(xt[kt], xT_d[kt * 128:(kt + 1) * 128, :])

        # padded K stationaries: kp0[pr] rows 0:64 = head 2pr K, rows 64: zero
        kp0 = [resid.tile([128, T], BF16, tag=f"kp0_{i}", name=f"kp0_{i}")
               for i in range(6)]
        kp1 = [resid.tile([128, T], BF16, tag=f"kp1_{i}", name=f"kp1_{i}")
               for i in range(6)]
        for pr in range(6):
            nc.gpsimd.memset(kp0[pr][64:128, :], 0.0)
            nc.gpsimd.memset(kp1[pr][0:64, :], 0.0)
        # V natural [tokens, 12 heads, 64+1] with ones column (full seq)
        v_aug = [resid.tile([128, H, HD + 1], BF16, tag=f"vaug{i}", name=f"vaug{i}")
                 for i in range(NTT)]
        for tt in range(NTT):
            nc.gpsimd.memset(v_aug[tt][:, :, HD:HD + 1], 1.0)

        mask_sb = singles.tile([128, NTT, TH], BF16)

        # LN1 of layer 0: stats directly after the x DMAs
        ln_next = LnState(tc, nc, "l0a", lnsp, small, ln_d[0][0], ln_d[0][1], xbfp)
        for kt in range(NKT):
            ln_next.stat_kt(nc, scratch, xt[kt])

        for l in range(L):
            with ExitStack() as lctx:
                lnpool = lctx.enter_context(tc.tile_pool(name=f"ln{l}", bufs=1))
                wpool = lctx.enter_context(tc.tile_pool(name=f"w{l}", bufs=3))
                biasp = lctx.enter_context(tc.tile_pool(name=f"bias{l}", bufs=1))

                qkb_sb = biasp.tile([128, 12], F32)
                nc.sync.dma_start(qkb_sb, qkb_d[l].rearrange("(t p) -> p t", p=128))
                vbbf_sb = biasp.tile([1, D], BF16)
                nc.sync.dma_start(vbbf_sb, vb_d[l].rearrange("(o d) -> o d", o=1))
                pb_sb = biasp.tile([128, 6], F32)
                nc.sync.dma_start(pb_sb, pb_d[l].rearrange("(t p) -> p t", p=128))
                fcb_sb = biasp.tile([128, 24], F32)
                nc.sync.dma_start(fcb_sb, fcb_d[l].rearrange("(t p) -> p t", p=128))
                fc2b_sb = biasp.tile([128, 6], F32)
                nc.sync.dma_start(fc2b_sb, fc2b_d[l].rearrange("(t p) -> p t", p=128))

                # ---------- LN1 finish ----------
                h_bf = [lnpool.tile([128, TH], BF16, tag=f"hbf{i}", name=f"hbf{i}")
                        for i in range(NKT)]
                with tc.tile_pool(name=f"ab{l}a", bufs=1, space="PSUM") as ab_ps:
                    ln_finish(tc, nc, ln_next, xt, h_bf, small, scratch, ab_ps)

                # ---------- K own-half -> staging -> AllGather ----------
                k_own = lnpool.tile([128, 6, TH], BF16, tag="k_own", name="k_own")
                kin_b = dram.tile([128, 6, TH], BF16, tag=f"kin{l}", name=f"kin{l}")
                kout_b = dram.tile([2, 128, 6, TH], BF16, tag=f"kout{l}",
                                   name=f"kout{l}")
                with tc.tile_pool(name=f"qkps{l}", bufs=3, space="PSUM") as qkps:
                    for pr in range(6):
                        f = 6 + pr
                        wt = wpool.tile([128, NKT, 128], BF16, tag="qkw_t",
                                        name="qkw_t")
                        nc.sync.dma_start(wt, qkw_d[l, f])
                        ps = qkps.tile([128, TH], F32, tag="qkps", name="qkps")
                        for kt in range(NKT):
                            nc.tensor.matmul(ps, wt[:, kt, :], h_bf[kt],
                                             start=(kt == 0), stop=(kt == NKT - 1))
                        nc.scalar.activation(k_own[:, pr, :], ps, AF.Identity,
                                             bias=qkb_sb[:, f:f + 1])
                    nc.sync.dma_start(kin_b, k_own)
                    nc.gpsimd.collective_compute(
                        "AllGather", ALU.bypass, replica_groups=RG,
                        ins=[kin_b[:]], outs=[kout_b[:]])

                    # ---------- V own-half -> staging -> AllGather ----------
                    v_own = lnpool.tile([128, NTH, D], BF16, tag="v_own",
                                        name="v_own")
                    vin_b = dram.tile([128, NTH, D], BF16, tag=f"vin{l}",
                                      name=f"vin{l}")
                    vout_b = dram.tile([2, 128, NTH, D], BF16, tag=f"vout{l}",
                                       name=f"vout{l}")
                    vw_sb = [wpool.tile([128, D], BF16, tag=f"vw{i}",
                                        name=f"vw{i}", bufs=1)
                             for i in range(NKT)]
                    for kt in range(NKT):
                        nc.sync.dma_start(vw_sb[kt],
                                          vw_d[l][kt * 128:(kt + 1) * 128, :])
                    for tt in range(NTH):
                        for vc in range(2):
                            vs = slice(vc * 384, (vc + 1) * 384)
                            ps = qkps.tile([128, 384], F32, tag="vps", name="vps")
                            for kt in range(NKT):
                                nc.tensor.matmul(
                                    ps, h_bf[kt][:, tt * 128:(tt + 1) * 128],
                                    vw_sb[kt][:, vs],
                                    start=(kt == 0), stop=False)
                            nc.tensor.matmul(ps, ones_row[:, 0:128],
                                             vbbf_sb[:, vs],
                                             start=False, stop=True)
                            nc.vector.tensor_copy(v_own[:, tt, vs], ps)
                    nc.sync.dma_start(vin_b, v_own)
                    nc.gpsimd.collective_compute(
                        "AllGather", ALU.bypass, replica_groups=RG,
                        ins=[vin_b[:]], outs=[vout_b[:]])

                    # ---------- Q own-half ----------
                    q_sb = [lnpool.tile([128, TH], BF16, tag=f"q{i}", name=f"q{i}")
                            for i in range(6)]
                    for pr in range(6):
                        wt = wpool.tile([128, NKT, 128], BF16, tag="qkw_t",
                                        name="qkw_t")
                        nc.sync.dma_start(wt, qkw_d[l, pr])
                        ps = qkps.tile([128, TH], F32, tag="qkps", name="qkps")
                        for kt in range(NKT):
                            nc.tensor.matmul(ps, wt[:, kt, :], h_bf[kt],
                                             start=(kt == 0), stop=(kt == NKT - 1))
                        nc.scalar.activation(q_sb[pr], ps, AF.Identity,
                                             bias=qkb_sb[:, pr:pr + 1])

                if l == 0:
                    # masks are first needed by the scores below; deferring the
                    # DMA keeps startup queues free for x/weights
                    nc.sync.dma_start(mask_sb, mask_d.rearrange("j p q -> p j q"))

                # K readback into padded stationaries (both halves, uniform)
                for pr in range(6):
                    for rk in range(2):
                        cs = slice(rk * TH, (rk + 1) * TH)
                        nc.sync.dma_start(kp0[pr][0:64, cs],
                                          kout_b[rk, 0:64, pr, :])
                        nc.sync.dma_start(kp1[pr][64:128, cs],
                                          kout_b[rk, 64:128, pr, :])
                # V readback (both halves)
                for tt in range(NTT):
                    nc.sync.dma_start(
                        v_aug[tt][:, :, 0:HD],
                        vout_b[tt // NTH, :, tt % NTH, :]
                        .rearrange("p (h d) -> p h d", d=HD))

                # ---------- attention per head-pair ----------
                attoT = [lnpool.tile([128, TH], BF16, tag=f"attoT{i}",
                                     name=f"attoT{i}")
                         for i in range(NKT)]
                with tc.tile_pool(name=f"sps{l}", bufs=3, space="PSUM") as sps, \
                     tc.tile_pool(name=f"ops{l}", bufs=2, space="PSUM") as ops, \
                     tc.tile_pool(name=f"bps{l}", bufs=1, space="PSUM") as bps, \
                     tc.tile_pool(name=f"attp{l}", bufs=2) as attp, \
                     tc.tile_pool(name=f"nrm{l}", bufs=1) as nrmp:
                    pending = []

                    def emit_norm(job):
                        pr_, hh_, raw_, rcp_ = job
                        rb = scratch.tile([1, TH], BF16, tag="rb", name="rb")
                        nc.vector.tensor_copy(rb, rcp_)
                        bc = bps.tile([64, TH], F32, tag="bc", name="bc")
                        nc.tensor.matmul(bc, ones_col128[:, 0:64], rb,
                                         start=True, stop=True)
                        bc_sb = scratch.tile([64, TH], BF16, tag="bc_sb",
                                             name="bc_sb")
                        nc.scalar.copy(bc_sb, bc)
                        nc.vector.tensor_mul(
                            attoT[pr_][hh_ * 64:(hh_ + 1) * 64, :], raw_, bc_sb)

                    for pr in range(6):
                        attT = [[attp.tile([128, TH], BF16, tag=f"attT{hh}_{kt}",
                                           name=f"attT{hh}_{kt}")
                                 for kt in range(NTT)] for hh in range(2)]
                        for kt in range(NTT):
                            ks = slice(kt * 128, (kt + 1) * 128)
                            for hh, kp in ((0, kp0), (1, kp1)):
                                ps = sps.tile([128, TH], F32, tag="sps",
                                              name="sps")
                                nc.tensor.matmul(ps, kp[pr][:, ks], q_sb[pr],
                                                 start=True, stop=True)
                                dst = attT[hh][kt]
                                nc.scalar.activation(dst, ps, AF.Exp,
                                                     scale=0.125)
                                nc.vector.tensor_mul(dst, dst,
                                                     mask_sb[:, kt, :])
                        for hh in range(2):
                            h = 2 * pr + hh
                            po = ops.tile([128, TH], F32, tag="ops", name="ops")
                            for kt in range(NTT):
                                nc.tensor.matmul(
                                    po[0:HD + 1, :],
                                    v_aug[kt][:, h, :], attT[hh][kt],
                                    start=(kt == 0), stop=(kt == NTT - 1))
                            raw = nrmp.tile([64, TH], BF16, tag=f"raw{hh}",
                                            name=f"raw{hh}", bufs=2)
                            nc.scalar.copy(raw, po[0:HD, :])
                            den = scratch.tile([1, TH], F32, tag="den",
                                               name="den")
                            rcp = nrmp.tile([1, TH], F32, tag=f"rcp{hh}",
                                            name=f"rcp{hh}", bufs=2)
                            nc.vector.tensor_copy(den, po[HD:HD + 1, :])
                            nc.vector.reciprocal_approx_fast(rcp, den)
                            pending.append((pr, hh, raw, rcp))
                        if pr >= 1:
                            emit_norm(pending.pop(0))
                            emit_norm(pending.pop(0))
                    for job in pending:
                        emit_norm(job)

                # ---------- proj + residual (LN2 stats interleaved) ----------
                pw_sb = [wpool.tile([128, D], BF16, tag=f"pw{i}", name=f"pw{i}",
                                    bufs=1)
                         for i in range(NKT)]
                for kt in range(NKT):
                    nc.sync.dma_start(pw_sb[kt], pw_d[l][kt * 128:(kt + 1) * 128, :])
                ln2 = LnState(tc, nc, f"l{l}b", lnsp, small,
                              ln_d[l][2], ln_d[l][3], xbfp)
                with tc.tile_pool(name=f"pps{l}", bufs=4, space="PSUM") as pps:
                    for ot in range(NKT):
                        ps = pps.tile([128, TH], F32, tag="pps", name="pps")
                        for kt in range(NKT):
                            nc.tensor.matmul(
                                ps, pw_sb[kt][:, ot * 128:(ot + 1) * 128],
                                attoT[kt],
                                start=(kt == 0), stop=(kt == NKT - 1))
                        nc.vector.scalar_tensor_tensor(
                            xt[ot], ps, pb_sb[:, ot:ot + 1],
                            xt[ot], op0=ALU.add, op1=ALU.add)
                        if ot >= 1:
                            ln2.stat_kt(nc, scratch, xt[ot - 1])
                    ln2.stat_kt(nc, scratch, xt[NKT - 1])

                # ---------- LN2 finish + MLP (next-LN stats interleaved) ----
                h2in = [lnpool.tile([128, TH], BF16, tag=f"h2bf{i}",
                                    name=f"h2bf{i}")
                        for i in range(NKT)]
                with tc.tile_pool(name=f"ab{l}b", bufs=1, space="PSUM") as ab_ps:
                    ln_finish(tc, nc, ln2, xt, h2in, small, scratch, ab_ps)

                if l + 1 < L:
                    ln_next = LnState(tc, nc, f"l{l + 1}a", lnsp, small,
                                      ln_d[l + 1][0], ln_d[l + 1][1], xbfp)
                else:
                    ln_next = LnState(tc, nc, "lf", lnsp, small,
                                      lnf_d[0], lnf_d[1], xbfp)
                with tc.tile_pool(name=f"mlpps{l}", bufs=3, space="PSUM") as mlpps, \
                     tc.tile_pool(name=f"h2p{l}", bufs=1) as h2p:
                    h2c = [h2p.tile([128, TH], BF16, tag=f"h2c{f}", name=f"h2c{f}")
                           for f in range(24)]
                    for f in range(24):
                        wt = wpool.tile([128, NKT, 128], BF16, tag="fcw_t",
                                        name="fcw_t")
                        nc.sync.dma_start(wt, fcw_d[l, f])
                        ps = mlpps.tile([128, TH], F32, tag="fcps", name="fcps")
                        for kt in range(NKT):
                            nc.tensor.matmul(ps, wt[:, kt, :], h2in[kt],
                                             start=(kt == 0), stop=(kt == NKT - 1))
                        nc.scalar.activation(h2c[f], ps, AF.Gelu_apprx_tanh,
                                             bias=fcb_sb[:, f:f + 1])
                    for ot in range(NKT):
                        wt = wpool.tile([128, 24, 128], BF16, tag="fc2w_t",
                                        name="fc2w_t", bufs=2)
                        nc.sync.dma_start(wt, fc2w_d[l, ot])
                        ps = mlpps.tile([128, TH], F32, tag="fc2ps", name="fc2ps")
                        for kt in range(24):
                            nc.tensor.matmul(ps, wt[:, kt, :], h2c[kt],
                                             start=(kt == 0), stop=(kt == 23))
                        nc.vector.scalar_tensor_tensor(
                            xt[ot], ps, fc2b_sb[:, ot:ot + 1],
                            xt[ot], op0=ALU.add, op1=ALU.add)
                        if ot >= 1:
                            ln_next.stat_kt(nc, scratch, xt[ot - 1])
                    ln_next.stat_kt(nc, scratch, xt[NKT - 1])

        # ---------- final LN + AllGather + lm_head ----------
        with ExitStack() as fctx:
            lnpool = fctx.enter_context(tc.tile_pool(name="lnfp", bufs=1))
            xf_own = [lnpool.tile([128, TH], BF16, tag=f"xfo{i}", name=f"xfo{i}")
                      for i in range(NKT)]
            lmwp = fctx.enter_context(tc.tile_pool(name="lmw", bufs=3))
            wt_pre = {}
            with tc.tile_pool(name="abf", bufs=1, space="PSUM") as ab_ps:
                ln_finish(tc, nc, ln_next, xt, xf_own, small, scratch, ab_ps)
            xin_b = dram.tile([128, NKT, TH], BF16, tag="xin", name="xin")
            xout_b = dram.tile([2, 128, NKT, TH], BF16, tag="xout", name="xout")
            for kt in range(NKT):
                nc.sync.dma_start(xin_b[:, kt, :], xf_own[kt])
            nc.gpsimd.collective_compute(
                "AllGather", ALU.bypass, replica_groups=RG,
                ins=[xin_b[:]], outs=[xout_b[:]])
            xf_bf = [lnpool.tile([128, T], BF16, tag=f"xf{i}", name=f"xf{i}")
                     for i in range(NKT)]
            for kt in range(NKT):
                for rk in range(2):
                    nc.sync.dma_start(xf_bf[kt][:, rk * TH:(rk + 1) * TH],
                                      xout_b[rk, :, kt, :])

            with tc.tile_pool(name="lmps", bufs=4, space="PSUM") as lmps, \
                 tc.tile_pool(name="lmev", bufs=4) as lmev:
                for vc in range(NVC):
                    if vc in wt_pre:
                        wt = wt_pre.pop(vc)
                    else:
                        wt = lmwp.tile([128, NKT, 512], BF16, tag="lmw_t",
                                       name="lmw_t")
                        nc.sync.dma_start(wt, lmw_d[vc])
                    for tt in range(NTT):
                        ps = lmps.tile([128, 512], F32, tag="lmps", name="lmps")
                        for kt in range(NKT):
                            nc.tensor.matmul(
                                ps, xf_bf[kt][:, tt * 128:(tt + 1) * 128],
                                wt[:, kt, :],
                                start=(kt == 0), stop=(kt == NKT - 1))
                        ev = lmev.tile([128, 512], F16, tag="lmev", name="lmev")
                        if tt % 2 == 0:
                            nc.scalar.copy(ev, ps)
                        else:
                            nc.vector.tensor_copy(ev, ps)
                        nc.sync.dma_start(
                            out_d[tt * 128:(tt + 1) * 128,
                                  vc * 512:(vc + 1) * 512], ev)
    nc.finalize()
    return nc


_NC_CACHE = None


def _get_nc():
    global _NC_CACHE
    if _NC_CACHE is None:
        _NC_CACHE = build_bass()
    return _NC_CACHE


def _block_w(w, no, nt):
    """[IN=nt*128, OUT=no*128] -> [no, 128, nt, 128] blocked for contiguous DMA:
    out[o, p, t, c] = w[t*128+p, o*128+c]."""
    IN, OUT = w.shape
    return np.ascontiguousarray(
        w.reshape(nt, 128, no, 128).transpose(2, 1, 0, 3))


def make_in_maps(idx, layer_num, wte, wpe, ln1_g, ln1_b, attn_w, attn_b, proj_w,
                 proj_b, ln2_g, ln2_b, fc_w, fc_b, fc2_w, fc2_b, lnf_g, lnf_b, lm_w):
    bf = ml_dtypes.bfloat16
    idx = np.asarray(idx)
    f32 = np.float32
    wte = np.asarray(wte, f32)
    wpe = np.asarray(wpe, f32)
    x0 = wte[idx] + wpe[:T]                      # [B,T,D] fp32 host embedding

    attn_w = np.asarray(attn_w, f32)
    qkw = np.stack([_block_w(attn_w[l, :, :2 * D], 12, NKT) for l in range(L)]
                   ).astype(bf)
    vw = np.ascontiguousarray(attn_w[:, :, 2 * D:]).astype(bf)
    pw = np.asarray(proj_w, f32).astype(bf)
    fcw = np.stack([_block_w(np.asarray(fc_w, f32)[l], 24, NKT)
                    for l in range(L)]).astype(bf)
    fc2w = np.stack([_block_w(np.asarray(fc2_w, f32)[l], 6, 24)
                     for l in range(L)]).astype(bf)
    qkb = np.ascontiguousarray(np.asarray(attn_b, f32)[:, :2 * D])
    vb = np.ascontiguousarray(np.asarray(attn_b, f32)[:, 2 * D:]).astype(bf)
    lnp = np.stack([np.asarray(ln1_g, f32), np.asarray(ln1_b, f32),
                    np.asarray(ln2_g, f32), np.asarray(ln2_b, f32)], axis=1)
    lnf = np.stack([np.asarray(lnf_g, f32), np.asarray(lnf_b, f32)], axis=0)

    lmw_pad = np.zeros((D, VPAD), f32)
    lmw_pad[:, :V] = np.asarray(lm_w, f32)

    # causal mask blocks per half h: mask[j][kk, qq] = (128*j + kk) <= (512*h + qq)
    jj = np.arange(NTT)[:, None, None] * 128 + np.arange(128)[None, :, None]
    qq = np.arange(TH)[None, None, :]
    masks = [(jj <= h * TH + qq).astype(bf) for h in range(2)]

    x0T = [np.ascontiguousarray(x0[b].T) for b in range(B)]
    # lm_w halves blocked: [NVC, 128, NKT, 512]; lmw[vc, p, t, c] =
    # w[t*128+p, vc*512+c]
    lmw_halves = []
    for h in range(2):
        wh = lmw_pad[:, h * VSH:(h + 1) * VSH]
        lmw_halves.append(np.ascontiguousarray(
            wh.reshape(NKT, 128, NVC, 512).transpose(2, 1, 0, 3)).astype(bf))

    in_maps = []
    for core in range(8):
        b = core // 2
        h = core % 2
        in_maps.append(dict(
            xT=np.ascontiguousarray(x0T[b][:, h * TH:(h + 1) * TH]),
            qkw=qkw, vw=vw, pw=pw, fcw=fcw, fc2w=fc2w,
            qkb=qkb, vb=vb, pb=np.asarray(proj_b, f32),
            fcb=np.asarray(fc_b, f32), fc2b=np.asarray(fc2_b, f32),
            lnp=lnp, lnf=lnf, mask=masks[h],
            lmw=lmw_halves[h],
        ))
    return in_maps


def kernel(**inputs):
    global LAST_RESULT
    in_maps = make_in_maps(**inputs)
    nc = _get_nc()
    res = run_bass_kernel_spmd(nc, in_maps, core_ids=list(range(8)), trace=TRACE)
    LAST_RESULT = res

    logits = np.empty((B, T, V), np.float32)
    for b in range(B):
        lo = res.results[2 * b]["out"].astype(np.float32)
        hi = res.results[2 * b + 1]["out"].astype(np.float32)
        logits[b, :, :VSH] = lo
        logits[b, :, VSH:] = hi[:, :V - VSH]
    return logits
